# revision 74
# baseline (speedup 1.0000x reference)
"""DigitCaps kernel for 8 Trainium2 NeuronCores.

Math (per batch b):
    U_hat[b,d,n,j] = sum_i W[d,n,j,i] * u[b,n,i]
    A_sum[b,d,m]   = sum_n U_hat[b,d,n,:] . U_hat[b,d,m,:] / sqrt(dp)
                   = s[b,d,:] . U_hat[b,d,m,:] / sqrt(dp),  s = sum_n U_hat
    C              = softmax_d(A_sum)
    S[b,d,j]       = sum_m (B_prior[d,m] + C[b,d,m]) * U_hat[b,d,m,j]
    out            = squash(S)

The huge [B,D,N,N] similarity matrix collapses because it is immediately
summed over n - only the n-sum s of U_hat is needed.

Sharding: data-parallel over batch, 2 batches per core, W/B_prior replicated.
Inputs are pre-arranged on the host into per-tile layouts so every device DMA
reads fully contiguous memory.

Per-core layout: n-tiles of 128 on partitions.
    W_sb[nt]  : [n=128, (d,j,i)=1280]   (host-arranged, contiguous 5KB rows)
    U2[nt]    : [n=128, (b,d,j)=320]    multiply-accumulate chain over i
                (DVE TensorScalarPtr; 4 half-chains on GpSimd as mult+tree)
    s         : ones.T @ U2 fp32        (PE partition-reduce, per-batch-half
                                         PSUM groups; all rows equal s)
    then per n-tile (pipelined): A_sum (GpSimd mult + DVE reduce) -> exp
    (ACT, scale=1/sqrt(dp), table preloaded) -> softmax_d norm (DVE) ->
    +B_prior -> S matmul (PE, bf16, PSUM accum);
    diagonal extract via iota mask; squash with Newton sqrt on DVE (keeps
    the Exp ACT table resident - no table reloads in the tail).
"""

import math
import numpy as np

import concourse.bacc as bacc
import concourse.bass as bass
import concourse.tile as tile
from concourse import mybir
from concourse.bass_utils import run_bass_kernel_spmd

F32 = mybir.dt.float32
I32 = mybir.dt.int32
AX = mybir.AxisListType
OP = mybir.AluOpType
ACTF = mybir.ActivationFunctionType

B, N, DP = 16, 1152, 8
D, DD = 10, 16
NCORES = 8
BPC = B // NCORES            # 2 batches per core
NT = N // 128                # 9 n-tiles
FW = D * DD * DP             # 1280 W free size
FD = D * DD                  # 160 per-batch U2 free size
FU = BPC * FD                # 320 U2 free size
NBD = BPC * D                # 20 (b,d) pairs
EPS = 1e-7
INV_SQRT_DP = 1.0 / math.sqrt(DP)


def _build_kernel(tc: "tile.TileContext", out_ap, WUB):
    nc = tc.nc
    with (
        tc.tile_pool(name="wpool", bufs=NT) as wpool,
        tc.tile_pool(name="tapool", bufs=6) as tapool,
        tc.tile_pool(name="ppool", bufs=4) as ppool,
        tc.tile_pool(name="smpool", bufs=2) as smpool,
        tc.tile_pool(name="persist", bufs=1) as persist,
        tc.tile_pool(name="psum_s", bufs=1, space="PSUM") as psum_s,
        tc.tile_pool(name="psum_S2", bufs=1, space="PSUM") as psum_S2,
    ):
        BF16 = mybir.dt.bfloat16
        ones_t = persist.tile([128, 128], F32, tag="ones")
        nc.vector.memset(ones_t[:], 1.0)

        u2_all = persist.tile([128, NT * FU], F32, tag="u2all")
        u2bf_all = persist.tile([128, NT * FU], BF16, tag="u2bfall")
        cbbf_all = persist.tile([128, NT * NBD], BF16, tag="cbbfall")
        cb_all = persist.tile([128, NT * NBD], F32, tag="cball")
        e_all = persist.tile([128, NT * NBD], F32, tag="eall")
        z_all = persist.tile([128, NT * BPC], F32, tag="zall")
        zr_all = persist.tile([128, NT * BPC], F32, tag="zrall")

        s_ps_0 = psum_s.tile([128, FD], F32, tag="sps0")
        s_ps_1 = psum_s.tile([128, FD], F32, tag="sps1")
        s_ps_b = [s_ps_0, s_ps_1]

        # preload the Exp ACT table while ACT is idle (hides the ~1.3us
        # table load that would otherwise land in the phase-2 critical path)
        warm_t = persist.tile([1, 1], F32, tag="warm")
        nc.vector.memset(warm_t[:], 0.0)
        nc.scalar.activation(warm_t[:], warm_t[:], ACTF.Exp)

        # ---- phase 1: load; U2 votes via i-chain; running s on PE ----
        ACT_TILES = ()  # these tiles route products via ACT + GpSimd
        # (nt, b) half-chains routed to GpSimd (mult + tree, all Pool-legal)
        POOL_HALVES = {(1, 1), (3, 1), (5, 1), (7, 1)}
        w_tiles = []
        for nt in range(NT):
            w_t = wpool.tile([128, FW + BPC * DP + D], F32, tag="w")
            w_tiles.append(w_t)
            nc.sync.dma_start(w_t[:], WUB[nt])
            u_t = w_t[:, FW: FW + BPC * DP]

            # U2[n,(b,d,j)] += W[n,(d,j,i)] * u[n,(b,i)]  accumulated over i.
            # TensorScalarPtr is DVE-only on trn2 (walrus rejects it on Pool),
            # so offload tiles via ACT products + GpSimd tree-reduce instead.
            w_3 = w_t[:, :FW].rearrange("p (dj i) -> p dj i", dj=FD, i=DP)
            if nt in ACT_TILES:
                # products P[n,(b,dj,i)] on ACT (Copy with per-partition
                # scale), then i-tree-reduce on GpSimd
                pp = ppool.tile([128, BPC * FW], F32, tag="pp")
                pp_v = pp[:].rearrange(
                    "p (b dj i) -> p b dj i", b=BPC, dj=FD, i=DP
                )
                for b in range(BPC):
                    for i in range(DP):
                        nc.scalar.activation(
                            pp_v[:, b, :, i],
                            w_3[:, :, i],
                            ACTF.Copy,
                            scale=u_t[:, b * DP + i: b * DP + i + 1],
                        )
                t1 = ppool.tile([128, BPC * FD * 4], F32, tag="t1")
                t1_v = t1[:].rearrange("p (g i) -> p g i", g=BPC * FD, i=4)
                pp_g = pp[:].rearrange("p (g i) -> p g i", g=BPC * FD, i=DP)
                nc.gpsimd.tensor_tensor(
                    t1_v, pp_g[:, :, 0:4], pp_g[:, :, 4:8], OP.add
                )
                t2 = ppool.tile([128, BPC * FD * 2], F32, tag="t2")
                t2_v = t2[:].rearrange("p (g i) -> p g i", g=BPC * FD, i=2)
                nc.gpsimd.tensor_tensor(
                    t2_v, t1_v[:, :, 0:2], t1_v[:, :, 2:4], OP.add
                )
                nc.gpsimd.tensor_tensor(
                    u2_all[:, nt * FU:(nt + 1) * FU].rearrange(
                        "p (g i) -> p g i", g=BPC * FD, i=1
                    ),
                    t2_v[:, :, 0:1],
                    t2_v[:, :, 1:2],
                    OP.add,
                )
            else:
                for b in range(BPC):
                    u2_sl = u2_all[:, nt * FU + b * FD: nt * FU + (b + 1) * FD]
                    if (nt, b) in POOL_HALVES:
                        # GpSimd route: one big mult + 3 tree-adds over i
                        pp = ppool.tile([128, FW], F32, tag="pp")
                        pp_v = pp[:].rearrange("p (g i) -> p g i", g=FD, i=DP)
                        u_bc = (
                            u_t[:, b * DP:(b + 1) * DP]
                            .unsqueeze(1)
                            .broadcast_to([128, FD, DP])
                        )
                        nc.gpsimd.tensor_tensor(pp_v, w_3, u_bc, OP.mult)
                        t1 = ppool.tile([128, FD * 4], F32, tag="t1")
                        t1_v = t1[:].rearrange("p (g i) -> p g i", g=FD, i=4)
                        nc.gpsimd.tensor_tensor(
                            t1_v, pp_v[:, :, 0:4], pp_v[:, :, 4:8], OP.add
                        )
                        t2 = ppool.tile([128, FD * 2], F32, tag="t2")
                        t2_v = t2[:].rearrange("p (g i) -> p g i", g=FD, i=2)
                        nc.gpsimd.tensor_tensor(
                            t2_v, t1_v[:, :, 0:2], t1_v[:, :, 2:4], OP.add
                        )
                        nc.gpsimd.tensor_tensor(
                            u2_sl.rearrange("p (g i) -> p g i", g=FD, i=1),
                            t2_v[:, :, 0:1],
                            t2_v[:, :, 1:2],
                            OP.add,
                        )
                        nc.tensor.matmul(
                            s_ps_b[b][:],
                            ones_t[:],
                            u2_sl,
                            start=(nt == 0),
                            stop=(nt == NT - 1),
                        )
                        continue
                    # first product on ACT (Copy with per-partition scale)
                    # frees two DVE ops per tile
                    nc.scalar.activation(
                        u2_sl,
                        w_3[:, :, 0],
                        ACTF.Copy,
                        scale=u_t[:, b * DP: b * DP + 1],
                    )
                    for i in range(1, DP):
                        nc.vector.scalar_tensor_tensor(
                            u2_sl,
                            w_3[:, :, i],
                            u_t[:, b * DP + i: b * DP + i + 1],
                            u2_sl,
                            OP.mult,
                            OP.add,
                        )
                    # s accumulation for this half-chain (fp32, PE idle;
                    # column-split groups give finer start dependencies)
                    nc.tensor.matmul(
                        s_ps_b[b][:],
                        ones_t[:],
                        u2_sl,
                        start=(nt == 0),
                        stop=(nt == NT - 1),
                    )



        # ---- phase 2 (pipelined per n-tile): A_sum -> softmax_d -> +B_prior
        #      -> S matmul ----
        # s copy to SBUF so GpSimd (no PSUM access) can read it (DVE: the
        # chain engine is free here and ACT's queue is backlogged)
        s_sb = persist.tile([128, FU], F32, tag="ssb")
        for b in range(BPC):
            nc.vector.tensor_copy(s_sb[:, b * FD:(b + 1) * FD], s_ps_b[b][:])

        # bf16 shadow of U2 for the S2 matmuls - cast lazily here, where ACT
        # is otherwise idle and off the phase-1 -> phase-2 critical path
        for nt in range(NT):
            nc.scalar.copy(
                u2bf_all[:, nt * FU:(nt + 1) * FU],
                u2_all[:, nt * FU:(nt + 1) * FU],
            )

        S2_ps = psum_S2.tile([NBD, FU], F32, tag="S2")
        POOL_TILES = (2, 3, 4, 5, 6, 7, 8)  # TA on GpSimd for these n-tiles
        for nt in range(NT):
            u2_sl = u2_all[:, nt * FU:(nt + 1) * FU]
            a_sl = e_all[:, nt * NBD:(nt + 1) * NBD]  # staging (overwritten by exp)
            ta = tapool.tile([128, FU], F32, tag="ta")
            if nt in POOL_TILES:
                nc.gpsimd.tensor_tensor(ta[:], u2_sl, s_sb[:], OP.mult)
            else:
                nc.vector.tensor_tensor(ta[:], u2_sl, s_sb[:], OP.mult)
            nc.vector.tensor_reduce(
                a_sl,
                ta[:].rearrange("p (g j) -> p g j", g=NBD, j=DD),
                AX.X,
                OP.add,
            )
            # E = exp(A / sqrt(dp))
            nc.scalar.activation(a_sl, a_sl, ACTF.Exp, scale=INV_SQRT_DP)
            # z[(b)] = sum_d E ; zr = 1/z
            z_sl = z_all[:, nt * BPC:(nt + 1) * BPC]
            zr_sl = zr_all[:, nt * BPC:(nt + 1) * BPC]
            nc.vector.tensor_reduce(
                z_sl,
                a_sl.rearrange("p (b d) -> p b d", b=BPC, d=D),
                AX.X,
                OP.add,
            )
            nc.vector.reciprocal(zr_sl, z_sl)
            # cb = E * zr + B_prior, written directly as bf16 for the matmul
            cbbf_sl = cbbf_all[:, nt * NBD:(nt + 1) * NBD]
            for b in range(BPC):
                nc.vector.scalar_tensor_tensor(
                    cbbf_sl[:, b * D:(b + 1) * D],
                    a_sl[:, b * D:(b + 1) * D],
                    zr_sl[:, b: b + 1],
                    w_tiles[nt][:, FW + BPC * DP: FW + BPC * DP + D],
                    OP.mult,
                    OP.add,
                )
            # S2 += cb.T @ U2 (bf16 operands, fp32 PSUM accumulate)
            nc.tensor.matmul(
                S2_ps[:],
                cbbf_sl,
                u2bf_all[:, nt * FU:(nt + 1) * FU],
                start=(nt == 0),
                stop=(nt == NT - 1),
            )

        # ---- phase 3: extract diagonal (b,d)=(b',d') via iota mask ----
        iota_t = persist.tile([NBD, FU], I32, tag="iota")
        nc.gpsimd.iota(
            iota_t[:], pattern=[[1, NBD], [0, DD]], base=0, channel_multiplier=-1
        )
        mask_t = persist.tile([NBD, FU], F32, tag="mask")
        nc.vector.tensor_scalar(mask_t[:], iota_t[:], 0, None, OP.is_equal)

        sm_t = smpool.tile([NBD, FU], F32, tag="sm")
        nc.vector.tensor_tensor(sm_t[:], S2_ps[:], mask_t[:], OP.mult)
        s_diag = persist.tile([NBD, DD], F32, tag="sdiag")
        nc.vector.tensor_reduce(
            s_diag[:],
            sm_t[:].rearrange("p (g j) -> p j g", g=NBD, j=DD),
            AX.X,
            OP.add,
        )

        # ---- phase 4: squash ----
        ss_t = persist.tile([NBD, DD], F32, tag="ss")
        nrm2 = persist.tile([NBD, 1], F32, tag="nrm2")
        nc.vector.tensor_tensor(ss_t[:], s_diag[:], s_diag[:], OP.mult)
        nc.vector.tensor_reduce(nrm2[:], ss_t[:], AX.X, OP.add)
        # norm via DVE Newton sqrt (bit-hack seed + 2 iterations) - keeps the
        # Exp ACT table resident (no sqrt/exp table reload in the tail)
        nrm = persist.tile([NBD, 1], F32, tag="nrm")
        seed_i = persist.tile([NBD, 1], I32, tag="seedi")
        nc.vector.tensor_scalar(
            seed_i[:], nrm2[:].bitcast(I32), 1, None, OP.logical_shift_right
        )
        nc.vector.tensor_scalar(seed_i[:], seed_i[:], 0x1FBD1DF5, None, OP.add)
        nc.vector.tensor_copy(nrm[:], seed_i[:].bitcast(F32))
        nwr = persist.tile([NBD, 1], F32, tag="nwr")
        nwt = persist.tile([NBD, 1], F32, tag="nwt")
        for _ in range(2):
            nc.vector.reciprocal(nwr[:], nrm[:])
            nc.vector.tensor_tensor(nwt[:], nrm2[:], nwr[:], OP.mult)
            nc.vector.tensor_tensor(nrm[:], nrm[:], nwt[:], OP.add)
            nc.vector.tensor_scalar(nrm[:], nrm[:], 0.5, None, OP.mult)
        en = persist.tile([NBD, 1], F32, tag="en")
        nc.scalar.activation(en[:], nrm[:], ACTF.Exp)
        en_eps = persist.tile([NBD, 1], F32, tag="eneps")
        nc.vector.tensor_scalar(en_eps[:], en[:], EPS, None, OP.add)
        r1 = persist.tile([NBD, 1], F32, tag="r1")
        nc.vector.reciprocal(r1[:], en_eps[:])
        coef = persist.tile([NBD, 1], F32, tag="coef")
        nc.vector.tensor_scalar(coef[:], r1[:], -1.0, 1.0, OP.mult, OP.add)
        nrm_eps = persist.tile([NBD, 1], F32, tag="nrmeps")
        nc.vector.tensor_scalar(nrm_eps[:], nrm[:], EPS, None, OP.add)
        r2 = persist.tile([NBD, 1], F32, tag="r2")
        nc.vector.reciprocal(r2[:], nrm_eps[:])
        fac = persist.tile([NBD, 1], F32, tag="fac")
        nc.vector.tensor_tensor(fac[:], coef[:], r2[:], OP.mult)

        res_t = persist.tile([NBD, DD], F32, tag="res")
        nc.vector.tensor_scalar(res_t[:], s_diag[:], fac[:], None, OP.mult)

        nc.sync.dma_start(out_ap.rearrange("b d j -> (b d) j"), res_t[:])


_CACHE: dict = {}


def _get_nc():
    if "nc" not in _CACHE:
        nc = bacc.Bacc(
            "TRN2", target_bir_lowering=False, debug=False, num_devices=NCORES
        )
        # host-pre-arranged: W, u and B_prior fused per tile so each tile is
        # ONE fully contiguous DMA (cols 0:1280 = W, 1280:1296 = u, 1296:1306 = bp)
        WUB = nc.dram_tensor(
            "wub_arr", [NT, 128, FW + BPC * DP + D], F32, kind="ExternalInput"
        ).ap()
        out = nc.dram_tensor("out", [BPC, D, DD], F32, kind="ExternalOutput").ap()
        with tile.TileContext(nc) as tc:
            _build_kernel(tc, out, WUB)
        nc.compile()
        _CACHE["nc"] = nc
    return _CACHE["nc"]


def _arrange(primary_caps, W, B_prior, core):
    """Host-side pre-arrangement into the exact SBUF tile layouts so every
    device DMA reads fully contiguous memory."""
    W = np.asarray(W, dtype=np.float32)
    Bp = np.asarray(B_prior, dtype=np.float32)
    pc = np.asarray(primary_caps, dtype=np.float32)
    w_arr = W.transpose(1, 0, 2, 3).reshape(NT, 128, FW)
    u_arr = (
        pc[core * BPC:(core + 1) * BPC]
        .transpose(1, 0, 2)
        .reshape(NT, 128, BPC * DP)
    )
    bp_arr = Bp[:, 0, :].T.reshape(NT, 128, D)
    return {
        "wub_arr": np.ascontiguousarray(
            np.concatenate([w_arr, u_arr, bp_arr], axis=2)
        )
    }


def _run(primary_caps, W, B_prior, trace=False, **kw):
    nc = _get_nc()
    in_maps = [
        _arrange(primary_caps, W, B_prior, c) for c in range(NCORES)
    ]
    res = run_bass_kernel_spmd(nc, in_maps, list(range(NCORES)), trace=trace, **kw)
    out = np.concatenate([res.results[c]["out"] for c in range(NCORES)], axis=0)
    return out.astype(np.float32), res


def kernel(primary_caps, W, B_prior):
    out, _ = _run(primary_caps, W, B_prior, trace=False)
    return out


# revision 77
# speedup vs baseline: 1.0065x; 1.0065x over previous
"""DigitCaps kernel for 8 Trainium2 NeuronCores.

Math (per batch b):
    U_hat[b,d,n,j] = sum_i W[d,n,j,i] * u[b,n,i]
    A_sum[b,d,m]   = sum_n U_hat[b,d,n,:] . U_hat[b,d,m,:] / sqrt(dp)
                   = s[b,d,:] . U_hat[b,d,m,:] / sqrt(dp),  s = sum_n U_hat
    C              = softmax_d(A_sum)
    S[b,d,j]       = sum_m (B_prior[d,m] + C[b,d,m]) * U_hat[b,d,m,j]
    out            = squash(S)

The huge [B,D,N,N] similarity matrix collapses because it is immediately
summed over n - only the n-sum s of U_hat is needed.

Sharding: data-parallel over batch, 2 batches per core, W/B_prior replicated.
Inputs are pre-arranged on the host into per-tile layouts so every device DMA
reads fully contiguous memory.

Per-core layout: n-tiles of 128 on partitions.
    W_sb[nt]  : [n=128, (d,j,i)=1280]   (host-arranged, contiguous 5KB rows)
    U2[nt]    : [n=128, (b,d,j)=320]    multiply-accumulate chain over i
                (DVE TensorScalarPtr; 4 half-chains on GpSimd as mult+tree)
    s         : ones.T @ U2 fp32        (PE partition-reduce, per-batch-half
                                         PSUM groups; all rows equal s)
    then per n-tile (pipelined): A_sum (GpSimd mult + DVE reduce) -> exp
    (ACT, scale=1/sqrt(dp), table preloaded) -> softmax_d norm (DVE) ->
    +B_prior -> S matmul (PE, bf16, PSUM accum);
    diagonal extract via iota mask; squash with Newton sqrt on DVE (keeps
    the Exp ACT table resident - no table reloads in the tail).
"""

import math
import numpy as np

import concourse.bacc as bacc
import concourse.bass as bass
import concourse.tile as tile
from concourse import mybir
from concourse.bass_utils import run_bass_kernel_spmd

F32 = mybir.dt.float32
I32 = mybir.dt.int32
AX = mybir.AxisListType
OP = mybir.AluOpType
ACTF = mybir.ActivationFunctionType

B, N, DP = 16, 1152, 8
D, DD = 10, 16
NCORES = 8
BPC = B // NCORES            # 2 batches per core
NT = N // 128                # 9 n-tiles
FW = D * DD * DP             # 1280 W free size
FD = D * DD                  # 160 per-batch U2 free size
FU = BPC * FD                # 320 U2 free size
NBD = BPC * D                # 20 (b,d) pairs
EPS = 1e-7
INV_SQRT_DP = 1.0 / math.sqrt(DP)


def _build_kernel(tc: "tile.TileContext", out_ap, WUB):
    nc = tc.nc
    with (
        tc.tile_pool(name="wpool", bufs=NT) as wpool,
        tc.tile_pool(name="tapool", bufs=6) as tapool,
        tc.tile_pool(name="ppool", bufs=4) as ppool,
        tc.tile_pool(name="smpool", bufs=2) as smpool,
        tc.tile_pool(name="persist", bufs=1) as persist,
        tc.tile_pool(name="psum_s", bufs=1, space="PSUM") as psum_s,
        tc.tile_pool(name="psum_S2", bufs=1, space="PSUM") as psum_S2,
    ):
        BF16 = mybir.dt.bfloat16
        ones_t = persist.tile([128, 128], F32, tag="ones")
        nc.vector.memset(ones_t[:], 1.0)

        u2_all = persist.tile([128, NT * FU], F32, tag="u2all")
        u2bf_all = persist.tile([128, NT * FU], BF16, tag="u2bfall")
        cbbf_all = persist.tile([128, NT * NBD], BF16, tag="cbbfall")
        cb_all = persist.tile([128, NT * NBD], F32, tag="cball")
        e_all = persist.tile([128, NT * NBD], F32, tag="eall")
        z_all = persist.tile([128, NT * BPC], F32, tag="zall")
        zr_all = persist.tile([128, NT * BPC], F32, tag="zrall")

        s_ps_0 = psum_s.tile([128, FD], F32, tag="sps0")
        s_ps_1 = psum_s.tile([128, FD], F32, tag="sps1")
        s_ps_b = [s_ps_0, s_ps_1]

        # preload the Exp ACT table while ACT is idle (hides the ~1.3us
        # table load that would otherwise land in the phase-2 critical path)
        warm_t = persist.tile([1, 1], F32, tag="warm")
        nc.vector.memset(warm_t[:], 0.0)
        nc.scalar.activation(warm_t[:], warm_t[:], ACTF.Exp)

        # ---- phase 1: load; U2 votes via i-chain; running s on PE ----
        ACT_TILES = ()  # these tiles route products via ACT + GpSimd
        # (nt, b) half-chains routed to GpSimd (mult + tree, all Pool-legal)
        POOL_HALVES = {(1, 1), (3, 1), (5, 1), (7, 1)}
        w_tiles = []
        for nt in range(NT):
            w_t = wpool.tile([128, FW + BPC * DP + D], F32, tag="w")
            w_tiles.append(w_t)
            nc.sync.dma_start(w_t[:], WUB[nt])
            u_t = w_t[:, FW: FW + BPC * DP]

            # U2[n,(b,d,j)] += W[n,(d,j,i)] * u[n,(b,i)]  accumulated over i.
            # TensorScalarPtr is DVE-only on trn2 (walrus rejects it on Pool),
            # so offload tiles via ACT products + GpSimd tree-reduce instead.
            w_3 = w_t[:, :FW].rearrange("p (dj i) -> p dj i", dj=FD, i=DP)
            if nt in ACT_TILES:
                # products P[n,(b,dj,i)] on ACT (Copy with per-partition
                # scale), then i-tree-reduce on GpSimd
                pp = ppool.tile([128, BPC * FW], F32, tag="pp")
                pp_v = pp[:].rearrange(
                    "p (b dj i) -> p b dj i", b=BPC, dj=FD, i=DP
                )
                for b in range(BPC):
                    for i in range(DP):
                        nc.scalar.activation(
                            pp_v[:, b, :, i],
                            w_3[:, :, i],
                            ACTF.Copy,
                            scale=u_t[:, b * DP + i: b * DP + i + 1],
                        )
                t1 = ppool.tile([128, BPC * FD * 4], F32, tag="t1")
                t1_v = t1[:].rearrange("p (g i) -> p g i", g=BPC * FD, i=4)
                pp_g = pp[:].rearrange("p (g i) -> p g i", g=BPC * FD, i=DP)
                nc.gpsimd.tensor_tensor(
                    t1_v, pp_g[:, :, 0:4], pp_g[:, :, 4:8], OP.add
                )
                t2 = ppool.tile([128, BPC * FD * 2], F32, tag="t2")
                t2_v = t2[:].rearrange("p (g i) -> p g i", g=BPC * FD, i=2)
                nc.gpsimd.tensor_tensor(
                    t2_v, t1_v[:, :, 0:2], t1_v[:, :, 2:4], OP.add
                )
                nc.gpsimd.tensor_tensor(
                    u2_all[:, nt * FU:(nt + 1) * FU].rearrange(
                        "p (g i) -> p g i", g=BPC * FD, i=1
                    ),
                    t2_v[:, :, 0:1],
                    t2_v[:, :, 1:2],
                    OP.add,
                )
            else:
                for b in range(BPC):
                    u2_sl = u2_all[:, nt * FU + b * FD: nt * FU + (b + 1) * FD]
                    if (nt, b) in POOL_HALVES:
                        # GpSimd route: one big mult + 3 tree-adds over i
                        pp = ppool.tile([128, FW], F32, tag="pp")
                        pp_v = pp[:].rearrange("p (g i) -> p g i", g=FD, i=DP)
                        u_bc = (
                            u_t[:, b * DP:(b + 1) * DP]
                            .unsqueeze(1)
                            .broadcast_to([128, FD, DP])
                        )
                        nc.gpsimd.tensor_tensor(pp_v, w_3, u_bc, OP.mult)
                        t1 = ppool.tile([128, FD * 4], F32, tag="t1")
                        t1_v = t1[:].rearrange("p (g i) -> p g i", g=FD, i=4)
                        nc.gpsimd.tensor_tensor(
                            t1_v, pp_v[:, :, 0:4], pp_v[:, :, 4:8], OP.add
                        )
                        t2 = ppool.tile([128, FD * 2], F32, tag="t2")
                        t2_v = t2[:].rearrange("p (g i) -> p g i", g=FD, i=2)
                        nc.gpsimd.tensor_tensor(
                            t2_v, t1_v[:, :, 0:2], t1_v[:, :, 2:4], OP.add
                        )
                        nc.gpsimd.tensor_tensor(
                            u2_sl.rearrange("p (g i) -> p g i", g=FD, i=1),
                            t2_v[:, :, 0:1],
                            t2_v[:, :, 1:2],
                            OP.add,
                        )
                        nc.tensor.matmul(
                            s_ps_b[b][:],
                            ones_t[:],
                            u2_sl,
                            start=(nt == 0),
                            stop=(nt == NT - 1),
                        )
                        continue
                    # first product on ACT (Copy with per-partition scale)
                    # frees two DVE ops per tile
                    nc.scalar.activation(
                        u2_sl,
                        w_3[:, :, 0],
                        ACTF.Copy,
                        scale=u_t[:, b * DP: b * DP + 1],
                    )
                    for i in range(1, DP):
                        nc.vector.scalar_tensor_tensor(
                            u2_sl,
                            w_3[:, :, i],
                            u_t[:, b * DP + i: b * DP + i + 1],
                            u2_sl,
                            OP.mult,
                            OP.add,
                        )
                    # s accumulation for this half-chain (fp32, PE idle;
                    # column-split groups give finer start dependencies)
                    nc.tensor.matmul(
                        s_ps_b[b][:],
                        ones_t[:],
                        u2_sl,
                        start=(nt == 0),
                        stop=(nt == NT - 1),
                    )



        # ---- phase 2 (pipelined per n-tile): A_sum -> softmax_d -> +B_prior
        #      -> S matmul ----
        # s copy to SBUF so GpSimd (no PSUM access) can read it (DVE: the
        # chain engine is free here and ACT's queue is backlogged)
        s_sb = persist.tile([128, FU], F32, tag="ssb")
        for b in range(BPC):
            nc.vector.tensor_copy(s_sb[:, b * FD:(b + 1) * FD], s_ps_b[b][:])

        # bf16 shadow of U2 for the S2 matmuls - cast lazily here, where ACT
        # is otherwise idle and off the phase-1 -> phase-2 critical path
        for nt in range(NT):
            nc.scalar.copy(
                u2bf_all[:, nt * FU:(nt + 1) * FU],
                u2_all[:, nt * FU:(nt + 1) * FU],
            )

        S2_ps = psum_S2.tile([NBD, FU], F32, tag="S2")
        POOL_TILES = (2, 3, 4, 5, 6, 7, 8)  # TA on GpSimd for these n-tiles
        for nt in range(NT):
            u2_sl = u2_all[:, nt * FU:(nt + 1) * FU]
            a_sl = e_all[:, nt * NBD:(nt + 1) * NBD]  # staging (overwritten by exp)
            ta = tapool.tile([128, FU], F32, tag="ta")
            if nt in POOL_TILES:
                nc.gpsimd.tensor_tensor(ta[:], u2_sl, s_sb[:], OP.mult)
            else:
                nc.vector.tensor_tensor(ta[:], u2_sl, s_sb[:], OP.mult)
            nc.vector.tensor_reduce(
                a_sl,
                ta[:].rearrange("p (g j) -> p g j", g=NBD, j=DD),
                AX.X,
                OP.add,
            )
            # E = exp(A / sqrt(dp))
            nc.scalar.activation(a_sl, a_sl, ACTF.Exp, scale=INV_SQRT_DP)
            # z[(b)] = sum_d E ; zr = 1/z
            z_sl = z_all[:, nt * BPC:(nt + 1) * BPC]
            zr_sl = zr_all[:, nt * BPC:(nt + 1) * BPC]
            nc.vector.tensor_reduce(
                z_sl,
                a_sl.rearrange("p (b d) -> p b d", b=BPC, d=D),
                AX.X,
                OP.add,
            )
            nc.vector.reciprocal(zr_sl, z_sl)
            # cb = E * zr + B_prior, written directly as bf16 for the matmul
            cbbf_sl = cbbf_all[:, nt * NBD:(nt + 1) * NBD]
            for b in range(BPC):
                nc.vector.scalar_tensor_tensor(
                    cbbf_sl[:, b * D:(b + 1) * D],
                    a_sl[:, b * D:(b + 1) * D],
                    zr_sl[:, b: b + 1],
                    w_tiles[nt][:, FW + BPC * DP: FW + BPC * DP + D],
                    OP.mult,
                    OP.add,
                )
            # S2 += cb.T @ U2 (bf16 operands, fp32 PSUM accumulate)
            nc.tensor.matmul(
                S2_ps[:],
                cbbf_sl,
                u2bf_all[:, nt * FU:(nt + 1) * FU],
                start=(nt == 0),
                stop=(nt == NT - 1),
            )

        # ---- phase 3: extract diagonal (b,d)=(b',d') via iota mask ----
        iota_t = persist.tile([NBD, FU], I32, tag="iota")
        nc.gpsimd.iota(
            iota_t[:], pattern=[[1, NBD], [0, DD]], base=0, channel_multiplier=-1
        )
        mask_t = persist.tile([NBD, FU], F32, tag="mask")
        nc.vector.tensor_scalar(mask_t[:], iota_t[:], 0, None, OP.is_equal)

        sm_t = smpool.tile([NBD, FU], F32, tag="sm")
        nc.vector.tensor_tensor(sm_t[:], S2_ps[:], mask_t[:], OP.mult)
        s_diag = persist.tile([NBD, DD], F32, tag="sdiag")
        nc.vector.tensor_reduce(
            s_diag[:],
            sm_t[:].rearrange("p (g j) -> p j g", g=NBD, j=DD),
            AX.X,
            OP.add,
        )

        # ---- phase 4: squash ----
        ss_t = persist.tile([NBD, DD], F32, tag="ss")
        nrm2 = persist.tile([NBD, 1], F32, tag="nrm2")
        nc.vector.tensor_tensor(ss_t[:], s_diag[:], s_diag[:], OP.mult)
        nc.vector.tensor_reduce(nrm2[:], ss_t[:], AX.X, OP.add)
        # norm via DVE Newton sqrt (bit-hack seed + 2 iterations) - keeps the
        # Exp ACT table resident (no sqrt/exp table reload in the tail)
        # norm via one Halley iteration from the bit-hack seed (cubic:
        # 3.5e-2 seed error -> ~4e-5), all on DVE
        nrm = persist.tile([NBD, 1], F32, tag="nrm")
        seed_i = persist.tile([NBD, 1], I32, tag="seedi")
        nc.vector.tensor_scalar(
            seed_i[:], nrm2[:].bitcast(I32), 1, None, OP.logical_shift_right
        )
        nc.vector.tensor_scalar(seed_i[:], seed_i[:], 0x1FBD1DF5, None, OP.add)
        nc.vector.tensor_copy(nrm[:], seed_i[:].bitcast(F32))
        y2 = persist.tile([NBD, 1], F32, tag="y2")
        nc.vector.tensor_tensor(y2[:], nrm[:], nrm[:], OP.mult)
        hnum = persist.tile([NBD, 1], F32, tag="hnum")
        nc.vector.scalar_tensor_tensor(hnum[:], nrm2[:], 3.0, y2[:], OP.mult, OP.add)
        hden = persist.tile([NBD, 1], F32, tag="hden")
        nc.vector.scalar_tensor_tensor(hden[:], y2[:], 3.0, nrm2[:], OP.mult, OP.add)
        nwr = persist.tile([NBD, 1], F32, tag="nwr")
        nc.vector.reciprocal(nwr[:], hden[:])
        nwt = persist.tile([NBD, 1], F32, tag="nwt")
        nc.vector.tensor_tensor(nwt[:], hnum[:], nwr[:], OP.mult)
        nc.vector.tensor_tensor(nrm[:], nrm[:], nwt[:], OP.mult)
        # coef = 1 - 1/(e^r + eps) ~= 1 - e^-r  (abs diff <= eps*e^-2r <= 1e-7)
        en = persist.tile([NBD, 1], F32, tag="en")
        nc.scalar.activation(en[:], nrm[:], ACTF.Exp, scale=-1.0)
        coef = persist.tile([NBD, 1], F32, tag="coef")
        nc.vector.tensor_scalar(coef[:], en[:], -1.0, 1.0, OP.mult, OP.add)
        nrm_eps = persist.tile([NBD, 1], F32, tag="nrmeps")
        nc.vector.tensor_scalar(nrm_eps[:], nrm[:], EPS, None, OP.add)
        r2 = persist.tile([NBD, 1], F32, tag="r2")
        nc.vector.reciprocal(r2[:], nrm_eps[:])
        fac = persist.tile([NBD, 1], F32, tag="fac")
        nc.vector.tensor_tensor(fac[:], coef[:], r2[:], OP.mult)

        res_t = persist.tile([NBD, DD], F32, tag="res")
        nc.vector.tensor_scalar(res_t[:], s_diag[:], fac[:], None, OP.mult)

        nc.sync.dma_start(out_ap.rearrange("b d j -> (b d) j"), res_t[:])


_CACHE: dict = {}


def _get_nc():
    if "nc" not in _CACHE:
        nc = bacc.Bacc(
            "TRN2", target_bir_lowering=False, debug=False, num_devices=NCORES
        )
        # host-pre-arranged: W, u and B_prior fused per tile so each tile is
        # ONE fully contiguous DMA (cols 0:1280 = W, 1280:1296 = u, 1296:1306 = bp)
        WUB = nc.dram_tensor(
            "wub_arr", [NT, 128, FW + BPC * DP + D], F32, kind="ExternalInput"
        ).ap()
        out = nc.dram_tensor("out", [BPC, D, DD], F32, kind="ExternalOutput").ap()
        with tile.TileContext(nc) as tc:
            _build_kernel(tc, out, WUB)
        nc.compile()
        _CACHE["nc"] = nc
    return _CACHE["nc"]


def _arrange(primary_caps, W, B_prior, core):
    """Host-side pre-arrangement into the exact SBUF tile layouts so every
    device DMA reads fully contiguous memory."""
    W = np.asarray(W, dtype=np.float32)
    Bp = np.asarray(B_prior, dtype=np.float32)
    pc = np.asarray(primary_caps, dtype=np.float32)
    w_arr = W.transpose(1, 0, 2, 3).reshape(NT, 128, FW)
    u_arr = (
        pc[core * BPC:(core + 1) * BPC]
        .transpose(1, 0, 2)
        .reshape(NT, 128, BPC * DP)
    )
    bp_arr = Bp[:, 0, :].T.reshape(NT, 128, D)
    return {
        "wub_arr": np.ascontiguousarray(
            np.concatenate([w_arr, u_arr, bp_arr], axis=2)
        )
    }


def _run(primary_caps, W, B_prior, trace=False, **kw):
    nc = _get_nc()
    in_maps = [
        _arrange(primary_caps, W, B_prior, c) for c in range(NCORES)
    ]
    res = run_bass_kernel_spmd(nc, in_maps, list(range(NCORES)), trace=trace, **kw)
    out = np.concatenate([res.results[c]["out"] for c in range(NCORES)], axis=0)
    return out.astype(np.float32), res


def kernel(primary_caps, W, B_prior):
    out, _ = _run(primary_caps, W, B_prior, trace=False)
    return out
